# revision 6
# baseline (speedup 1.0000x reference)
"""Trainium2 Bass kernel for nn_ATConv (dynamic per-(b,c) 3x3 depthwise conv
between two 1x1 convs, with a pooled-gelu kernel-generation branch).

Sharding: data-parallel over batch B=16 across 8 NeuronCores (2 images/core).
Each core processes its 2 images as 3 "planes" of 128 partitions:
  P0 = img0 channels 0:128, P1 = img1 channels 0:128,
  P2 = packed [img0 c128:192 | img1 c128:192].

v2 design notes (vs the 306us baseline):
  - x is fed as fp16 from host (halves input HBM traffic); outputs are fp16
    and the Wp bias is folded in on host (kills all output-bias work).
  - Phase A (Wx conv) uses 7 streams per range instead of 8: the xb->P2
    contribution is one block-diagonal matmul serving both images.
  - Phase C (depthwise) uses fused scalar_tensor_tensor (y += k*x_shift),
    one DVE pass per tap instead of scale+add.  Odd shifts read from xpo,
    an element-shifted copy of xpe, to keep 4B alignment for the DVE 2x
    mode.  Wrap-column garbage is subtracted by tiny strided fixup ops.
  - Pooling of x runs on DVE via tensor_scalar accum_out (4x mode) for
    P0/P2 during the idle startup window, and on ACT for P1.
  - Phase D mirrors phase A's 7-stream structure; evictions are pure
    fp32->fp16 copies (no bias).
"""
import numpy as np

import concourse.bacc as bacc
import concourse.mybir as mybir
import concourse.tile as tile
from concourse import bass_utils

dt = mybir.dt
Alu = mybir.AluOpType
Act = mybir.ActivationFunctionType

B, C, H, W = 16, 192, 96, 96
L = H * W            # 9216
K2 = 9
SEG = L // K2        # 1024
NCORES = 8
NRANGE = L // SEG    # 9
RT = 1024
INV_SQRT2 = float(1.0 / np.sqrt(2.0))

# tap index t = 3*(dh+1) + (dw+1); center tap = 4
TAPS = [(t // 3 - 1, t % 3 - 1) for t in range(9)]

_BUILT = {}


def build():
    nc = bacc.Bacc("TRN2", target_bir_lowering=False, debug=False,
                   num_devices=NCORES)

    f16, f32 = dt.float16, dt.float32
    x0 = nc.dram_tensor("x0", [C, L], f16, kind="ExternalInput").ap()
    x1 = nc.dram_tensor("x1", [C, L], f16, kind="ExternalInput").ap()
    # phase A / D weight tiles (see host prep in kernel())
    wnames = ["a1", "a2_0", "a2_1", "a3", "a4",
              "p1", "p2_0", "p2_1", "p3", "p4"]
    wdram = {}
    for nm in wnames:
        cols = 64 if nm in ("a3", "p3") else 128
        wdram[nm] = nc.dram_tensor(f"w_{nm}", [128, cols], f16,
                                   kind="ExternalInput").ap()
    wkT_a = nc.dram_tensor("wkT_a", [128, 192], f16, kind="ExternalInput").ap()
    wkT_b0 = nc.dram_tensor("wkT_b0", [128, 192], f16, kind="ExternalInput").ap()
    wkT_b1 = nc.dram_tensor("wkT_b1", [128, 192], f16, kind="ExternalInput").ap()
    wg2 = nc.dram_tensor("wg2", [9, 9], f16, kind="ExternalInput").ap()
    bx_a = nc.dram_tensor("bx_a", [128, 1], f32, kind="ExternalInput").ap()
    bx_b = nc.dram_tensor("bx_b", [128, 1], f32, kind="ExternalInput").ap()
    dc_a = nc.dram_tensor("dc_a", [128, 1], f32, kind="ExternalInput").ap()
    dc_b = nc.dram_tensor("dc_b", [128, 1], f32, kind="ExternalInput").ap()
    bk_bc = nc.dram_tensor("bk_bc", [9, 192], f32, kind="ExternalInput").ap()
    bg_bc = nc.dram_tensor("bg_bc", [128, 9], f32, kind="ExternalInput").ap()
    out0 = nc.dram_tensor("out0", [C, L], f16, kind="ExternalOutput").ap()
    out1 = nc.dram_tensor("out1", [C, L], f16, kind="ExternalOutput").ap()

    PL = ["P0", "P1", "P2"]

    with tile.TileContext(nc) as tc:
        with tc.tile_pool(name="wpool", bufs=1) as wp, \
             tc.tile_pool(name="xin", bufs=1) as xin, \
             tc.tile_pool(name="small", bufs=1) as sm, \
             tc.tile_pool(name="xppool", bufs=1) as xpp, \
             tc.tile_pool(name="ypool", bufs=1) as yp, \
             tc.tile_pool(name="xopool", bufs=2) as xop, \
             tc.tile_pool(name="scr", bufs=2) as scr, \
             tc.tile_pool(name="stage", bufs=3) as stg:

            # ---- input x tiles ----
            xa0 = xin.tile([128, L], f16, tag="xa0", name="xa0")
            xa1 = xin.tile([128, L], f16, tag="xa1", name="xa1")
            xb = xin.tile([128, L], f16, tag="xb", name="xb")
            Q = L // 3
            for qi in range(3):
                lo_, hi_ = qi * Q, (qi + 1) * Q
                nc.gpsimd.dma_start(xa0[:, lo_:hi_], x0[0:128, lo_:hi_])
                nc.gpsimd.dma_start(xb[0:64, lo_:hi_], x0[128:192, lo_:hi_])
                nc.gpsimd.dma_start(xb[64:128, lo_:hi_], x1[128:192, lo_:hi_])
            for qi in range(3):
                lo_, hi_ = qi * Q, (qi + 1) * Q
                nc.gpsimd.dma_start(xa1[:, lo_:hi_], x1[0:128, lo_:hi_])

            # ---- persistent weights / biases ----
            wt = {}
            for nm in wnames:
                cols = 64 if nm in ("a3", "p3") else 128
                wt[nm] = wp.tile([128, cols], f16, tag=f"w_{nm}", name=f"w_{nm}")
                nc.sync.dma_start(wt[nm][:], wdram[nm][:, :])
            wka = wp.tile([128, 192], f16, tag="wka", name="wka")
            wkb0 = wp.tile([128, 192], f16, tag="wkb0", name="wkb0")
            wkb1 = wp.tile([128, 192], f16, tag="wkb1", name="wkb1")
            wgt = wp.tile([9, 9], f16, tag="wgt", name="wgt")
            for tl, src in [(wka, wkT_a), (wkb0, wkT_b0), (wkb1, wkT_b1),
                            (wgt, wg2)]:
                nc.sync.dma_start(tl[:], src[:, :])
            bias = {}
            for nm, src in [("bx_a", bx_a), ("bx_b", bx_b),
                            ("dc_a", dc_a), ("dc_b", dc_b)]:
                tl = wp.tile([128, 1], f32, tag=nm, name=nm)
                nc.sync.dma_start(tl[:], src[:, :])
                bias[nm] = tl
            bkb = wp.tile([9, 192], f32, tag="bkb", name="bkb")
            nc.sync.dma_start(bkb[:], bk_bc[:, :])
            bgb = wp.tile([128, 9], f32, tag="bgb", name="bgb")
            nc.sync.dma_start(bgb[:], bg_bc[:, :])

            factor = {}
            for p, srcn in [("P0", "dc_a"), ("P2", "dc_b")]:
                f = sm.tile([128, 1], f32, tag=f"factor{p}", name=f"factor{p}")
                nc.scalar.activation(f[:], bias[srcn][:], Act.Sigmoid,
                                     scale=1.0, bias=0.0)
                f9 = sm.tile([128, 1], f32, tag=f"f9{p}", name=f"f9{p}")
                nc.vector.tensor_scalar(f9[:], f[:], 1.0 / 9, None, Alu.mult)
                factor[p] = f9
            factor["P1"] = factor["P0"]

            biasx = {"P0": bias["bx_a"], "P1": bias["bx_a"], "P2": bias["bx_b"]}

            xpe = {p: xpp.tile([128, L], f16, tag=f"xpe{p}", name=f"xpe{p}")
                   for p in PL}
            pool = {p: sm.tile([128, 9], f32, tag=f"pool{p}", name=f"pool{p}")
                    for p in PL}

            # ---- pooling of x ----
            # P0/P2 on DVE (tensor_scalar 4x + accum_out) during startup;
            # P1 on ACT (off the critical path).
            def pool_dve(p, xt):
                for sj in range(9):
                    s0, s1 = sj * SEG, (sj + 1) * SEG
                    s = scr.tile([128, SEG], f16, tag="pscr", name="pscr")
                    nc.vector.tensor_scalar(s[:], xt[:, s0:s1], 1.0, 0.0,
                                            Alu.mult, Alu.add,
                                            accum_out=pool[p][:, sj:sj + 1])

            def pool_act(p, xt):
                for sj in range(9):
                    s0, s1 = sj * SEG, (sj + 1) * SEG
                    s = scr.tile([128, SEG], f16, tag="pscra", name="pscra")
                    nc.scalar.activation(s[:], xt[:, s0:s1], Act.Copy,
                                         accum_out=pool[p][:, sj:sj + 1])

            y_store = {"P0": yp.tile([128, L], f16, tag="yP0", name="yP0")}

            kfin = {}
            kneg = {}

            # ---------- phase A: xp = Wx x + bx ----------
            def a_plane(p):
                for r in range(NRANGE):
                    l0 = r * RT
                    ps = psA.tile([128, RT], f32, tag="psA", name="psA")
                    for (n0, n1) in [(l0, l0 + 512), (l0 + 512, l0 + RT)]:
                        d0, d1 = n0 - l0, n1 - l0
                        if p == "P0":
                            nc.tensor.matmul(ps[:, d0:d1], wt["a1"][:],
                                             xa0[:, n0:n1],
                                             start=True, stop=False)
                            nc.tensor.matmul(ps[:, d0:d1], wt["a2_0"][:],
                                             xb[:, n0:n1],
                                             start=False, stop=True)
                        elif p == "P1":
                            nc.tensor.matmul(ps[:, d0:d1], wt["a1"][:],
                                             xa1[:, n0:n1],
                                             start=True, stop=False)
                            nc.tensor.matmul(ps[:, d0:d1], wt["a2_1"][:],
                                             xb[:, n0:n1],
                                             start=False, stop=True)
                        else:
                            nc.tensor.matmul(ps[0:64, d0:d1], wt["a3"][:],
                                             xa0[:, n0:n1],
                                             start=True, stop=False)
                            nc.tensor.matmul(ps[64:128, d0:d1], wt["a3"][:],
                                             xa1[:, n0:n1],
                                             start=True, stop=False,
                                             tile_position=(0, 64))
                            nc.tensor.matmul(ps[:, d0:d1], wt["a4"][:],
                                             xb[:, n0:n1],
                                             start=False, stop=True)
                    nc.scalar.activation(xpe[p][:, l0:l0 + RT], ps[:],
                                         Act.Identity, bias=biasx[p][:])

            # ---------- kernel generation ----------
            psBd = {}
            pool16 = {}
            g16 = {}
            k9ps = {}

            def kg_pool16(p):
                t16 = sm.tile([128, 9], f16, tag=f"pool16{p}",
                              name=f"pool16{p}")
                nc.vector.tensor_scalar(t16[:], pool[p][:], 1.0 / SEG,
                                        None, Alu.mult)
                pool16[p] = t16

            def kg_img(i):
                pa = "P0" if i == 0 else "P1"
                kg_pool16(pa)
                if i == 0:
                    kg_pool16("P2")
                k1 = psBd["p"].tile([9, 192], f32, tag="k1T", name="k1T")
                wkbz = wkb0 if i == 0 else wkb1
                nc.tensor.matmul(k1[:], pool16[pa][:], wka[:],
                                 start=True, stop=False)
                nc.tensor.matmul(k1[:], pool16["P2"][:, :], wkbz[:, :],
                                 start=False, stop=True)
                s = sm.tile([9, 192], f32, tag=f"sB{i}", name=f"sB{i}")
                nc.vector.tensor_tensor(s[:], k1[:], bkb[:], Alu.add)
                e = sm.tile([9, 192], f32, tag=f"eB{i}", name=f"eB{i}")
                nc.scalar.activation(e[:], s[:], Act.Erf, scale=INV_SQRT2)
                g = sm.tile([9, 192], f16, tag=f"gB{i}", name=f"gB{i}")
                nc.vector.scalar_tensor_tensor(g[:], e[:], 1.0, s[:],
                                               Alu.add, Alu.mult)
                g16[i] = g

            def kg_fin(p):
                kb = sm.tile([128, 9], f32, tag=f"kb{p}", name=f"kb{p}")
                ms = sm.tile([128, 1], f32, tag=f"ms{p}", name=f"ms{p}")
                nc.vector.scalar_tensor_tensor(
                    kb[:], k9ps[p][:], 1.0, bgb[:], Alu.mult, Alu.add,
                    accum_out=ms[:])
                m2 = sm.tile([128, 1], f32, tag=f"m2{p}", name=f"m2{p}")
                nc.vector.tensor_scalar(m2[:], ms[:], factor[p][:], None,
                                        Alu.mult)
                kf = sm.tile([128, 9], f32, tag=f"kfin{p}", name=f"kfin{p}")
                nc.vector.tensor_scalar(kf[:], kb[:], m2[:], None,
                                        Alu.subtract)
                kfin[p] = kf
                kn = sm.tile([128, 9], f32, tag=f"kneg{p}", name=f"kneg{p}")
                nc.vector.tensor_scalar(kn[:], kf[:], -1.0, None, Alu.mult)
                kneg[p] = kn

            def kgen0():
                kg_img(0)
                k9ps["P0"] = psBd["p"].tile([128, 9], f32, tag="k9P0",
                                            name="k9P0")
                k9ps["P2"] = psBd["p"].tile([128, 9], f32, tag="k9P2",
                                            name="k9P2")
                nc.tensor.matmul(k9ps["P0"][:], g16[0][:, 0:128], wgt[:],
                                 start=True, stop=True)
                nc.tensor.matmul(k9ps["P2"][0:64, :], g16[0][:, 128:192],
                                 wgt[:], start=True, stop=True)
                kg_fin("P0")

            def kgen1():
                kg_img(1)
                k9ps["P1"] = psBd["p"].tile([128, 9], f32, tag="k9P1",
                                            name="k9P1")
                nc.tensor.matmul(k9ps["P1"][:], g16[1][:, 0:128], wgt[:],
                                 start=True, stop=True)
                nc.tensor.matmul(k9ps["P2"][64:128, :], g16[1][:, 128:192],
                                 wgt[:], start=True, stop=True,
                                 tile_position=(0, 64))
                kg_fin("P1")
                kg_fin("P2")

            # ---------- phase C: fused depthwise taps ----------
            def make_xpo(p):
                """xpo[l] = xpe[p][l-1]; cols 0 and L+1 stay zero."""
                xpo = xop.tile([128, L + 2], f16, tag="xpo", name="xpo")
                nc.vector.memset(xpo[:, 0:1], 0.0)
                nc.vector.memset(xpo[:, L + 1:L + 2], 0.0)
                nc.sync.dma_start(xpo[:, 1:1 + L], xpe[p][:, 0:L])
                return xpo

            def phase_c_dve(p, xpo, h0, h1):
                """Fused taps for plane p over output rows [h0, h1)."""
                y = y_store[p]
                kf = kfin[p]
                # center tap: y = k4 * xpe  (tensor_scalar, 4x mode)
                nc.vector.tensor_scalar(y[:, h0 * W:h1 * W],
                                        xpe[p][:, h0 * W:h1 * W],
                                        kf[:, 4:5], None, Alu.mult)
                for t in [1, 7, 3, 5, 0, 2, 6, 8]:
                    dh, dw = TAPS[t]
                    r0 = max(h0, -dh)
                    r1 = min(h1, H - dh)
                    if dw == 0:
                        src = xpe[p][:, (r0 + dh) * W:(r1 + dh) * W]
                    else:
                        b0 = (r0 + dh) * W + dw + 1
                        src = xpo[:, b0:b0 + (r1 - r0) * W]
                    nc.vector.scalar_tensor_tensor(
                        y[:, r0 * W:r1 * W], src, kf[:, t:t + 1],
                        y[:, r0 * W:r1 * W], Alu.mult, Alu.add)

            def phase_c_fixups(p, h0, h1):
                """Subtract wrap-column garbage for dw=+-1 taps."""
                y3 = y_store[p][:].rearrange("c (h w) -> c h w", h=H)
                xe3 = xpe[p][:].rearrange("c (h w) -> c h w", h=H)
                kn = kneg[p]
                for t in (2, 5, 8):          # dw = +1: wrong at col W-1
                    dh, _ = TAPS[t]
                    r0 = max(h0, -dh)
                    r1 = min(h1, H - dh, H - 1 - dh)
                    if r1 <= r0:
                        continue
                    nc.vector.scalar_tensor_tensor(
                        y3[:, r0:r1, W - 1:W],
                        xe3[:, r0 + dh + 1:r1 + dh + 1, 0:1],
                        kn[:, t:t + 1],
                        y3[:, r0:r1, W - 1:W], Alu.mult, Alu.add)
                for t in (0, 3, 6):          # dw = -1: wrong at col 0
                    dh, _ = TAPS[t]
                    r0 = max(h0, -dh, 1 - dh)
                    r1 = min(h1, H - dh)
                    if r1 <= r0:
                        continue
                    nc.vector.scalar_tensor_tensor(
                        y3[:, r0:r1, 0:1],
                        xe3[:, r0 + dh - 1:r1 + dh - 1, W - 1:W],
                        kn[:, t:t + 1],
                        y3[:, r0:r1, 0:1], Alu.mult, Alu.add)

            # ---------- phase D: out = Wp y (bias folded on host) ----------
            def d_img0(rlo, rhi):
                for r in range(rlo, rhi):
                    l0 = r * RT
                    pa = psD.tile([128, RT], f32, tag="psDa", name="psDa")
                    for (n0, n1) in [(0, 512), (512, RT)]:
                        nc.tensor.matmul(pa[:, n0:n1], wt["p1"][:],
                                         y_store["P0"][:, l0 + n0:l0 + n1],
                                         start=True, stop=False)
                        nc.tensor.matmul(pa[:, n0:n1], wt["p2_0"][:],
                                         y_store["P2"][:, l0 + n0:l0 + n1],
                                         start=False, stop=True)
                    sta = stg.tile([128, RT], f16, tag="sta", name="sta")
                    nc.scalar.activation(sta[:], pa[:], Act.Copy)
                    nc.sync.dma_start(out0[0:128, l0:l0 + RT], sta[:])

            def d_img1(rlo, rhi):
                for r in range(rlo, rhi):
                    l0 = r * RT
                    pa = psD.tile([128, RT], f32, tag="psDa", name="psDa")
                    p2 = psD.tile([128, RT], f32, tag="psD2", name="psD2")
                    for (n0, n1) in [(0, 512), (512, RT)]:
                        nc.tensor.matmul(pa[:, n0:n1], wt["p1"][:],
                                         y_store["P1"][:, l0 + n0:l0 + n1],
                                         start=True, stop=False)
                        nc.tensor.matmul(pa[:, n0:n1], wt["p2_1"][:],
                                         y_store["P2"][:, l0 + n0:l0 + n1],
                                         start=False, stop=True)
                        nc.tensor.matmul(p2[0:64, n0:n1], wt["p3"][:],
                                         y_store["P0"][:, l0 + n0:l0 + n1],
                                         start=True, stop=False)
                        nc.tensor.matmul(p2[64:128, n0:n1], wt["p3"][:],
                                         y_store["P1"][:, l0 + n0:l0 + n1],
                                         start=True, stop=False,
                                         tile_position=(0, 64))
                        nc.tensor.matmul(p2[:, n0:n1], wt["p4"][:],
                                         y_store["P2"][:, l0 + n0:l0 + n1],
                                         start=False, stop=True)
                    sta = stg.tile([128, RT], f16, tag="stb", name="stb")
                    nc.scalar.activation(sta[:], pa[:], Act.Copy)
                    nc.sync.dma_start(out1[0:128, l0:l0 + RT], sta[:])
                    st2 = stg.tile([128, RT], f16, tag="st2", name="st2")
                    nc.scalar.activation(st2[:], p2[:], Act.Copy)
                    nc.sync.dma_start(out0[128:192, l0:l0 + RT], st2[0:64, :])
                    nc.sync.dma_start(out1[128:192, l0:l0 + RT],
                                      st2[64:128, :])

            # ================= schedule =================
            with tc.tile_pool(name="psA", bufs=2, space="PSUM") as psA, \
                 tc.tile_pool(name="psB", bufs=1, space="PSUM") as _psB:
                psBd["p"] = _psB
                pool_dve("P0", xa0)
                pool_dve("P2", xb)
                a_plane("P0")
                kgen0()
                a_plane("P2")
                pool_act("P1", xa1)
                xpo0 = make_xpo("P0")
                phase_c_dve("P0", xpo0, 0, H)
                phase_c_fixups("P0", 0, H)
                a_plane("P1")
                kgen1()

            with tc.tile_pool(name="psD", bufs=2, space="PSUM") as psD:
                y_store["P2"] = xa0
                xpo2 = make_xpo("P2")
                phase_c_dve("P2", xpo2, 0, H)
                phase_c_fixups("P2", 0, H)
                d_img0(0, 5)
                y_store["P1"] = xa1
                xpo1 = make_xpo("P1")
                phase_c_dve("P1", xpo1, 0, H)
                phase_c_fixups("P1", 0, H)
                d_img0(5, NRANGE)
                d_img1(0, NRANGE)

    nc.compile()
    return nc


def _get_nc():
    if "nc" not in _BUILT:
        _BUILT["nc"] = build()
    return _BUILT["nc"]


def _host_prep(Wk, bk, Wg, bg, Wx, bx, Wp, bp, dc):
    f32 = lambda a: np.ascontiguousarray(np.asarray(a, dtype=np.float32))
    f16T = lambda a: np.ascontiguousarray(
        np.asarray(a, dtype=np.float32).T.astype(np.float16))
    WxT = f16T(Wx)     # [c, o]
    WpT = f16T(Wp)
    WkT = f16T(Wk)
    z64 = np.zeros((64, 128), np.float16)

    def streams(WT, pre):
        blk = WT[128:192, 128:192]
        bd = np.zeros((128, 128), np.float16)
        bd[0:64, 0:64] = blk
        bd[64:128, 64:128] = blk
        return {
            f"w_{pre}1": np.ascontiguousarray(WT[0:128, 0:128]),
            f"w_{pre}2_0": np.ascontiguousarray(
                np.concatenate([WT[128:192, 0:128], z64], axis=0)),
            f"w_{pre}2_1": np.ascontiguousarray(
                np.concatenate([z64, WT[128:192, 0:128]], axis=0)),
            f"w_{pre}3": np.ascontiguousarray(WT[0:128, 128:192]),
            f"w_{pre}4": bd,
        }

    colv = lambda v, lo, hi: np.ascontiguousarray(
        np.asarray(v, dtype=np.float32)[lo:hi].reshape(-1, 1))
    dup_col = lambda v: np.ascontiguousarray(
        np.concatenate([colv(v, 128, 192), colv(v, 128, 192)], axis=0))
    zpad0 = lambda wT: np.ascontiguousarray(np.concatenate(
        [wT[128:192], np.zeros((64, 192), wT.dtype)], axis=0))
    zpad1 = lambda wT: np.ascontiguousarray(np.concatenate(
        [np.zeros((64, 192), wT.dtype), wT[128:192]], axis=0))

    shared = {}
    shared.update(streams(WxT, "a"))
    shared.update(streams(WpT, "p"))
    shared.update({
        "wkT_a": np.ascontiguousarray(WkT[0:128]),
        "wkT_b0": zpad0(WkT), "wkT_b1": zpad1(WkT),
        "wg2": np.ascontiguousarray(
            (0.5 * np.asarray(Wg, dtype=np.float32)).T.astype(np.float16)),
        "bx_a": colv(bx, 0, 128), "bx_b": dup_col(bx),
        "dc_a": colv(dc, 0, 128), "dc_b": dup_col(dc),
        "bk_bc": np.ascontiguousarray(np.tile(f32(bk).reshape(1, C), (9, 1))),
        "bg_bc": np.ascontiguousarray(np.tile(f32(bg).reshape(1, 9), (128, 1))),
    })
    return shared


def kernel(x, Wk, bk, Wg, bg, Wx, bx, Wp, bp, dc):
    nc = _get_nc()
    x16 = np.asarray(x, dtype=np.float32).reshape(B, C, L).astype(np.float16)
    shared = _host_prep(Wk, bk, Wg, bg, Wx, bx, Wp, bp, dc)
    in_maps = []
    for core in range(NCORES):
        m = dict(shared)
        m["x0"] = np.ascontiguousarray(x16[2 * core])
        m["x1"] = np.ascontiguousarray(x16[2 * core + 1])
        in_maps.append(m)

    res = bass_utils.run_bass_kernel_spmd(nc, in_maps,
                                          core_ids=list(range(NCORES)))
    bpc = np.asarray(bp, dtype=np.float32).reshape(1, C, 1)
    out = np.empty((B, C, H, W), dtype=np.float32)
    for core in range(NCORES):
        o0 = res.results[core]["out0"].astype(np.float32) + bpc[0]
        o1 = res.results[core]["out1"].astype(np.float32) + bpc[0]
        out[2 * core] = o0.reshape(C, H, W)
        out[2 * core + 1] = o1.reshape(C, H, W)
    return out


# revision 12
# speedup vs baseline: 1.6646x; 1.6646x over previous
"""Trainium2 Bass kernel for nn_ATConv (dynamic per-(b,c) 3x3 depthwise conv
between two 1x1 convs, with a pooled-gelu kernel-generation branch).

Sharding: data-parallel over batch B=16 across 8 NeuronCores (2 images/core).
Each core processes its 2 images as 3 "planes" of 128 partitions:
  P0 = img0 channels 0:128, P1 = img1 channels 0:128,
  P2 = packed [img0 c128:192 | img1 c128:192].

v2 design notes (vs the 306us baseline):
  - x is fed as fp16 from host (halves input HBM traffic); outputs are fp16
    and the Wp bias is folded in on host (kills all output-bias work).
  - Phase A (Wx conv) uses 7 streams per range instead of 8: the xb->P2
    contribution is one block-diagonal matmul serving both images.
  - Phase C (depthwise) uses fused scalar_tensor_tensor (y += k*x_shift),
    one DVE pass per tap instead of scale+add.  Odd shifts read from xpo,
    an element-shifted copy of xpe, to keep 4B alignment for the DVE 2x
    mode.  Wrap-column garbage is subtracted by tiny strided fixup ops.
  - Pooling of x runs on DVE via tensor_scalar accum_out (4x mode) for
    P0/P2 during the idle startup window, and on ACT for P1.
  - Phase D mirrors phase A's 7-stream structure; evictions are pure
    fp32->fp16 copies (no bias).
"""
import numpy as np

import concourse.bacc as bacc
import concourse.mybir as mybir
import concourse.tile as tile
from concourse import bass_utils

dt = mybir.dt
Alu = mybir.AluOpType
Act = mybir.ActivationFunctionType

B, C, H, W = 16, 192, 96, 96
L = H * W            # 9216
K2 = 9
SEG = L // K2        # 1024
NCORES = 8
NRANGE = L // SEG    # 9
RT = 1024
INV_SQRT2 = float(1.0 / np.sqrt(2.0))

# tap index t = 3*(dh+1) + (dw+1); center tap = 4
TAPS = [(t // 3 - 1, t % 3 - 1) for t in range(9)]
# rows [0, R_DVE[p]) of plane p run on DVE; the rest on PE (diag matmuls)
R_DVE = {"P0": 56, "P1": 32, "P2": 40}
ACT_TAPS = (1,)   # taps whose scale-copy runs on ACT instead of DVE

_BUILT = {}


def build():
    nc = bacc.Bacc("TRN2", target_bir_lowering=False, debug=False,
                   num_devices=NCORES)

    f16, f32 = dt.float16, dt.float32
    x0 = nc.dram_tensor("x0", [C, L], f16, kind="ExternalInput").ap()
    x1 = nc.dram_tensor("x1", [C, L], f16, kind="ExternalInput").ap()
    # phase A / D weight tiles (see host prep in kernel())
    wnames = ["a1", "a2_0", "a2_1", "a3", "a4",
              "p1", "p2_0", "p2_1", "p3", "p4"]
    wdram = {}
    for nm in wnames:
        cols = 64 if nm in ("a3", "p3") else 128
        wdram[nm] = nc.dram_tensor(f"w_{nm}", [128, cols], f16,
                                   kind="ExternalInput").ap()
    wkT_a = nc.dram_tensor("wkT_a", [128, 192], f16, kind="ExternalInput").ap()
    wkT_b0 = nc.dram_tensor("wkT_b0", [128, 192], f16, kind="ExternalInput").ap()
    wkT_b1 = nc.dram_tensor("wkT_b1", [128, 192], f16, kind="ExternalInput").ap()
    wg2 = nc.dram_tensor("wg2", [9, 9], f16, kind="ExternalInput").ap()
    bx_a = nc.dram_tensor("bx_a", [128, 1], f32, kind="ExternalInput").ap()
    bx_b = nc.dram_tensor("bx_b", [128, 1], f32, kind="ExternalInput").ap()
    dc_a = nc.dram_tensor("dc_a", [128, 1], f32, kind="ExternalInput").ap()
    dc_b = nc.dram_tensor("dc_b", [128, 1], f32, kind="ExternalInput").ap()
    bk_bc = nc.dram_tensor("bk_bc", [9, 192], f32, kind="ExternalInput").ap()
    bg_bc = nc.dram_tensor("bg_bc", [128, 9], f32, kind="ExternalInput").ap()
    ident = nc.dram_tensor("ident", [128, 128], f16, kind="ExternalInput").ap()
    out0 = nc.dram_tensor("out0", [C, L], f16, kind="ExternalOutput").ap()
    out1 = nc.dram_tensor("out1", [C, L], f16, kind="ExternalOutput").ap()

    PL = ["P0", "P1", "P2"]

    with tile.TileContext(nc) as tc:
        with tc.tile_pool(name="wpool", bufs=1) as wp, \
             tc.tile_pool(name="xin", bufs=1) as xin, \
             tc.tile_pool(name="small", bufs=1) as sm, \
             tc.tile_pool(name="xppool", bufs=1) as xpp, \
             tc.tile_pool(name="ypool", bufs=1) as yp, \
             tc.tile_pool(name="xopool", bufs=2) as xop, \
             tc.tile_pool(name="scr", bufs=2) as scr, \
             tc.tile_pool(name="dg", bufs=2) as dg, \
             tc.tile_pool(name="tp", bufs=2) as tp, \
             tc.tile_pool(name="stage", bufs=2) as stg:

            # ---- input x tiles ----
            xa0 = xin.tile([128, L], f16, tag="xa0", name="xa0")
            xa1 = xin.tile([128, L], f16, tag="xa1", name="xa1")
            xb = xin.tile([128, L], f16, tag="xb", name="xb")
            Q = L // 3
            for qi in range(3):
                lo_, hi_ = qi * Q, (qi + 1) * Q
                nc.gpsimd.dma_start(xa0[:, lo_:hi_], x0[0:128, lo_:hi_])
                nc.gpsimd.dma_start(xb[0:64, lo_:hi_], x0[128:192, lo_:hi_])
                nc.gpsimd.dma_start(xb[64:128, lo_:hi_], x1[128:192, lo_:hi_])
            for qi in range(3):
                lo_, hi_ = qi * Q, (qi + 1) * Q
                nc.gpsimd.dma_start(xa1[:, lo_:hi_], x1[0:128, lo_:hi_])

            # ---- persistent weights / biases ----
            wt = {}
            for nm in wnames:
                cols = 64 if nm in ("a3", "p3") else 128
                wt[nm] = wp.tile([128, cols], f16, tag=f"w_{nm}", name=f"w_{nm}")
                nc.sync.dma_start(wt[nm][:], wdram[nm][:, :])
            wka = wp.tile([128, 192], f16, tag="wka", name="wka")
            wkb0 = wp.tile([128, 192], f16, tag="wkb0", name="wkb0")
            wkb1 = wp.tile([128, 192], f16, tag="wkb1", name="wkb1")
            wgt = wp.tile([9, 9], f16, tag="wgt", name="wgt")
            for tl, src in [(wka, wkT_a), (wkb0, wkT_b0), (wkb1, wkT_b1),
                            (wgt, wg2)]:
                nc.sync.dma_start(tl[:], src[:, :])
            bias = {}
            for nm, src in [("bx_a", bx_a), ("bx_b", bx_b),
                            ("dc_a", dc_a), ("dc_b", dc_b)]:
                tl = wp.tile([128, 1], f32, tag=nm, name=nm)
                nc.sync.dma_start(tl[:], src[:, :])
                bias[nm] = tl
            bkb = wp.tile([9, 192], f32, tag="bkb", name="bkb")
            nc.sync.dma_start(bkb[:], bk_bc[:, :])
            bgb = wp.tile([128, 9], f32, tag="bgb", name="bgb")
            nc.sync.dma_start(bgb[:], bg_bc[:, :])

            factor = {}
            for p, srcn in [("P0", "dc_a"), ("P2", "dc_b")]:
                f = sm.tile([128, 1], f32, tag=f"factor{p}", name=f"factor{p}")
                nc.scalar.activation(f[:], bias[srcn][:], Act.Sigmoid,
                                     scale=1.0, bias=0.0)
                f9 = sm.tile([128, 1], f32, tag=f"f9{p}", name=f"f9{p}")
                nc.vector.tensor_scalar(f9[:], f[:], 1.0 / 9, None, Alu.mult)
                factor[p] = f9
            factor["P1"] = factor["P0"]

            biasx = {"P0": bias["bx_a"], "P1": bias["bx_a"], "P2": bias["bx_b"]}

            # xpe tiles carry 2 zero guard elements on each side (data at
            # offset 2) so shifted reads at the plane edges see zeros.
            xpe = {p: xpp.tile([128, L + 4], f16, tag=f"xpe{p}",
                               name=f"xpe{p}")
                   for p in PL}
            for p in PL:
                nc.vector.memset(xpe[p][:, 0:2], 0.0)
                nc.vector.memset(xpe[p][:, L + 2:L + 4], 0.0)
            pool = {p: sm.tile([128, 9], f32, tag=f"pool{p}", name=f"pool{p}")
                    for p in PL}

            # ---- pooling of x ----
            # P0/P2 on DVE (tensor_scalar 4x + accum_out) during startup;
            # P1 on ACT (off the critical path).
            def pool_dve(p, xt):
                for sj in range(9):
                    s0, s1 = sj * SEG, (sj + 1) * SEG
                    s = scr.tile([128, SEG], f16, tag="pscr", name="pscr")
                    nc.vector.tensor_scalar(s[:], xt[:, s0:s1], 1.0, 0.0,
                                            Alu.mult, Alu.add,
                                            accum_out=pool[p][:, sj:sj + 1])

            def pool_act(p, xt):
                for sj in range(9):
                    s0, s1 = sj * SEG, (sj + 1) * SEG
                    s = scr.tile([128, SEG], f16, tag="pscr", name="pscra")
                    nc.scalar.activation(s[:], xt[:, s0:s1], Act.Copy,
                                         accum_out=pool[p][:, sj:sj + 1])

            y_store = {"P0": yp.tile([128, L], f16, tag="yP0", name="yP0")}

            kfin = {}
            kneg = {}

            # ---------- phase A: xp = Wx x + bx ----------
            def a_plane(p):
                for r in range(NRANGE):
                    l0 = r * RT
                    ps = psA.tile([128, RT], f32, tag="psA", name="psA")
                    for (n0, n1) in [(l0, l0 + 512), (l0 + 512, l0 + RT)]:
                        d0, d1 = n0 - l0, n1 - l0
                        if p == "P0":
                            nc.tensor.matmul(ps[:, d0:d1], wt["a1"][:],
                                             xa0[:, n0:n1],
                                             start=True, stop=False)
                            nc.tensor.matmul(ps[:, d0:d1], wt["a2_0"][:],
                                             xb[:, n0:n1],
                                             start=False, stop=True)
                        elif p == "P1":
                            nc.tensor.matmul(ps[:, d0:d1], wt["a1"][:],
                                             xa1[:, n0:n1],
                                             start=True, stop=False)
                            nc.tensor.matmul(ps[:, d0:d1], wt["a2_1"][:],
                                             xb[:, n0:n1],
                                             start=False, stop=True)
                        else:
                            nc.tensor.matmul(ps[0:64, d0:d1], wt["a3"][:],
                                             xa0[:, n0:n1],
                                             start=True, stop=False)
                            nc.tensor.matmul(ps[64:128, d0:d1], wt["a3"][:],
                                             xa1[:, n0:n1],
                                             start=True, stop=False,
                                             tile_position=(0, 64))
                            nc.tensor.matmul(ps[:, d0:d1], wt["a4"][:],
                                             xb[:, n0:n1],
                                             start=False, stop=True)
                    nc.scalar.activation(xpe[p][:, 2 + l0:2 + l0 + RT],
                                         ps[:], Act.Identity,
                                         bias=biasx[p][:])

            # ---------- kernel generation ----------
            psBd = {}
            pool16 = {}
            g16 = {}
            k9ps = {}

            def kg_pool16(p):
                t16 = sm.tile([128, 9], f16, tag=f"pool16{p}",
                              name=f"pool16{p}")
                nc.vector.tensor_scalar(t16[:], pool[p][:], 1.0 / SEG,
                                        None, Alu.mult)
                pool16[p] = t16

            def kg_img(i):
                pa = "P0" if i == 0 else "P1"
                kg_pool16(pa)
                if i == 0:
                    kg_pool16("P2")
                k1 = psBd["p"].tile([9, 192], f32, tag="k1T", name="k1T")
                wkbz = wkb0 if i == 0 else wkb1
                nc.tensor.matmul(k1[:], pool16[pa][:], wka[:],
                                 start=True, stop=False)
                nc.tensor.matmul(k1[:], pool16["P2"][:, :], wkbz[:, :],
                                 start=False, stop=True)
                s = sm.tile([9, 192], f32, tag=f"sB{i}", name=f"sB{i}")
                nc.vector.tensor_tensor(s[:], k1[:], bkb[:], Alu.add)
                e = sm.tile([9, 192], f32, tag=f"eB{i}", name=f"eB{i}")
                nc.scalar.activation(e[:], s[:], Act.Erf, scale=INV_SQRT2)
                g = sm.tile([9, 192], f16, tag=f"gB{i}", name=f"gB{i}")
                nc.vector.scalar_tensor_tensor(g[:], e[:], 1.0, s[:],
                                               Alu.add, Alu.mult)
                g16[i] = g

            def kg_fin(p):
                kb = sm.tile([128, 9], f32, tag=f"kb{p}", name=f"kb{p}")
                ms = sm.tile([128, 1], f32, tag=f"ms{p}", name=f"ms{p}")
                nc.vector.scalar_tensor_tensor(
                    kb[:], k9ps[p][:], 1.0, bgb[:], Alu.mult, Alu.add,
                    accum_out=ms[:])
                m2 = sm.tile([128, 1], f32, tag=f"m2{p}", name=f"m2{p}")
                nc.vector.tensor_scalar(m2[:], ms[:], factor[p][:], None,
                                        Alu.mult)
                kf = sm.tile([128, 9], f32, tag=f"kfin{p}", name=f"kfin{p}")
                nc.vector.tensor_scalar(kf[:], kb[:], m2[:], None,
                                        Alu.subtract)
                kfin[p] = kf
                kn = sm.tile([128, 9], f32, tag=f"kneg{p}", name=f"kneg{p}")
                nc.vector.tensor_scalar(kn[:], kf[:], -1.0, None, Alu.mult)
                kneg[p] = kn

            def kgen0():
                kg_img(0)
                k9ps["P0"] = psBd["p"].tile([128, 9], f32, tag="k9P0",
                                            name="k9P0")
                k9ps["P2"] = psBd["p"].tile([128, 9], f32, tag="k9P2",
                                            name="k9P2")
                nc.tensor.matmul(k9ps["P0"][:], g16[0][:, 0:128], wgt[:],
                                 start=True, stop=True)
                nc.tensor.matmul(k9ps["P2"][0:64, :], g16[0][:, 128:192],
                                 wgt[:], start=True, stop=True)
                kg_fin("P0")

            def kgen1():
                kg_img(1)
                k9ps["P1"] = psBd["p"].tile([128, 9], f32, tag="k9P1",
                                            name="k9P1")
                nc.tensor.matmul(k9ps["P1"][:], g16[1][:, 0:128], wgt[:],
                                 start=True, stop=True)
                nc.tensor.matmul(k9ps["P2"][64:128, :], g16[1][:, 128:192],
                                 wgt[:], start=True, stop=True,
                                 tile_position=(0, 64))
                kg_fin("P1")
                kg_fin("P2")

            # ---------- phase C: depthwise taps ----------
            # Per plane: rows [0, R_DVE[p]) on DVE/ACT (scale via
            # tensor_scalar 4x into scratch + tensor_tensor 2x add), rows
            # [R_DVE[p], 96) on PE (diagonal matmuls accumulating in PSUM,
            # ACT evicts fp32->fp16).  Wrap columns fixed up afterwards.
            XPO_ROWS = max(R_DVE.values()) + 1
            XPO_SZ = XPO_ROWS * W + 2
            TSCR_SZ = max(R_DVE.values()) * W

            def make_xpo(p):
                """xpo[l] = xpe[p][l-1] over the DVE region (+1 halo row)."""
                xpo = xop.tile([128, XPO_SZ], f16, tag="xpo", name="xpo")
                nc.vector.memset(xpo[:, 0:1], 0.0)
                n = min(R_DVE[p] + 1, H) * W + 1   # +1: tap8 corner read
                nc.sync.dma_start(xpo[:, 1:1 + n], xpe[p][:, 2:2 + n])
                return xpo

            diag = {}

            def build_diag(p):
                dd = []
                for t in range(9):
                    dtl = dg.tile([128, 128], f16, tag=f"diag{t}",
                                  name=f"diag{t}")
                    nc.vector.tensor_scalar(dtl[:], ident_t[:],
                                            kfin[p][:, t:t + 1], None,
                                            Alu.mult)
                    dd.append(dtl)
                diag[p] = dd

            def phase_c_dve(p, xpo, h0, h1):
                """Scale+add taps for plane p over output rows [h0, h1)."""
                if h1 <= h0:
                    return
                y = y_store[p]
                kf = kfin[p]
                # center tap: y = k4 * xpe  (tensor_scalar, 4x mode)
                nc.vector.tensor_scalar(y[:, h0 * W:h1 * W],
                                        xpe[p][:, 2 + h0 * W:2 + h1 * W],
                                        kf[:, 4:5], None, Alu.mult)
                for t in [1, 7, 3, 5, 0, 2, 6, 8]:
                    dh, dw = TAPS[t]
                    r0 = max(h0, -dh)
                    r1 = min(h1, H - dh)
                    n = (r1 - r0) * W
                    if dw == 0:
                        src = xpe[p][:, 2 + (r0 + dh) * W:2 + (r1 + dh) * W]
                    else:
                        b0 = (r0 + dh) * W + dw + 1
                        src = xpo[:, b0:b0 + n]
                    ts = tp.tile([128, TSCR_SZ], f16, tag="tscr",
                                 name="tscr")
                    if t in ACT_TAPS:
                        nc.scalar.activation(ts[:, 0:n], src, Act.Copy,
                                             scale=kf[:, t:t + 1])
                    else:
                        nc.vector.tensor_scalar(ts[:, 0:n], src,
                                                kf[:, t:t + 1], None,
                                                Alu.mult)
                    nc.vector.tensor_tensor(y[:, r0 * W:r1 * W], ts[:, 0:n],
                                            y[:, r0 * W:r1 * W], Alu.add)

            def _pieces(a, b, step):
                out = []
                while a < b:
                    nxt = min(b, (a // step + 1) * step)
                    out.append((a, nxt))
                    a = nxt
                return out

            def phase_c_pe(p, h0, h1):
                """Diag-matmul taps for plane p over rows [h0, h1)."""
                if h1 <= h0:
                    return
                y = y_store[p]
                dd = diag[p]
                l0 = h0 * W
                lend = h1 * W
                while l0 < lend:
                    csz = min(RT, lend - l0)
                    pc = psC.tile([128, RT], f32, tag="psC", name="psC")
                    for t in [4, 0, 1, 2, 3, 6, 7, 8, 5]:
                        dh, dw = TAPS[t]
                        at = max(0, -dh) * W
                        bt = (H - max(0, dh)) * W
                        a = max(l0, at)
                        b = min(l0 + csz, bt)
                        if b <= a:
                            continue
                        s = dh * W + dw
                        for (m0, m1) in _pieces(a - l0, b - l0, 512):
                            nc.tensor.matmul(
                                pc[:, m0:m1], dd[t][:],
                                xpe[p][:, 2 + l0 + m0 + s:2 + l0 + m1 + s],
                                start=(t == 4), stop=(t == 5))
                    nc.scalar.activation(y[:, l0:l0 + csz], pc[:, 0:csz],
                                         Act.Copy)
                    l0 += csz

            def phase_c_fixups(p, h0, h1):
                """Subtract wrap-column garbage for dw=+-1 taps."""
                y3 = y_store[p][:].rearrange("c (h w) -> c h w", h=H)
                xe3 = xpe[p][:, 2:2 + L].rearrange("c (h w) -> c h w",
                                                   h=H)
                kn = kneg[p]
                for t in (2, 5, 8):          # dw = +1: wrong at col W-1
                    dh, _ = TAPS[t]
                    r0 = max(h0, -dh)
                    r1 = min(h1, H - dh, H - 1 - dh)
                    if r1 <= r0:
                        continue
                    nc.vector.scalar_tensor_tensor(
                        y3[:, r0:r1, W - 1:W],
                        xe3[:, r0 + dh + 1:r1 + dh + 1, 0:1],
                        kn[:, t:t + 1],
                        y3[:, r0:r1, W - 1:W], Alu.mult, Alu.add)
                for t in (0, 3, 6):          # dw = -1: wrong at col 0
                    dh, _ = TAPS[t]
                    r0 = max(h0, -dh, 1 - dh)
                    r1 = min(h1, H - dh)
                    if r1 <= r0:
                        continue
                    nc.vector.scalar_tensor_tensor(
                        y3[:, r0:r1, 0:1],
                        xe3[:, r0 + dh - 1:r1 + dh - 1, W - 1:W],
                        kn[:, t:t + 1],
                        y3[:, r0:r1, 0:1], Alu.mult, Alu.add)

            # ---------- phase D: out = Wp y (bias folded on host) ----------
            def d_img0():
                for r in range(NRANGE):
                    l0 = r * RT
                    pa = psD.tile([128, RT], f32, tag="psDa", name="psDa")
                    for (n0, n1) in [(0, 512), (512, RT)]:
                        nc.tensor.matmul(pa[:, n0:n1], wt["p1"][:],
                                         y_store["P0"][:, l0 + n0:l0 + n1],
                                         start=True, stop=False)
                        nc.tensor.matmul(pa[:, n0:n1], wt["p2_0"][:],
                                         y_store["P2"][:, l0 + n0:l0 + n1],
                                         start=False, stop=True)
                    sta = stg.tile([128, RT], f16, tag="sta", name="sta")
                    nc.scalar.activation(sta[:], pa[:], Act.Copy)
                    nc.sync.dma_start(out0[0:128, l0:l0 + RT], sta[:])

            def d_img1():
                for r in range(NRANGE):
                    l0 = r * RT
                    pa = psD.tile([128, RT], f32, tag="psDa", name="psDa")
                    p2 = psD.tile([128, RT], f32, tag="psD2", name="psD2")
                    for (n0, n1) in [(0, 512), (512, RT)]:
                        nc.tensor.matmul(pa[:, n0:n1], wt["p1"][:],
                                         y_store["P1"][:, l0 + n0:l0 + n1],
                                         start=True, stop=False)
                        nc.tensor.matmul(pa[:, n0:n1], wt["p2_1"][:],
                                         y_store["P2"][:, l0 + n0:l0 + n1],
                                         start=False, stop=True)
                        nc.tensor.matmul(p2[0:64, n0:n1], wt["p3"][:],
                                         y_store["P0"][:, l0 + n0:l0 + n1],
                                         start=True, stop=False)
                        nc.tensor.matmul(p2[64:128, n0:n1], wt["p3"][:],
                                         y_store["P1"][:, l0 + n0:l0 + n1],
                                         start=True, stop=False,
                                         tile_position=(0, 64))
                        nc.tensor.matmul(p2[:, n0:n1], wt["p4"][:],
                                         y_store["P2"][:, l0 + n0:l0 + n1],
                                         start=False, stop=True)
                    sta = stg.tile([128, RT], f16, tag="stb", name="stb")
                    nc.scalar.activation(sta[:], pa[:], Act.Copy)
                    nc.sync.dma_start(out1[0:128, l0:l0 + RT], sta[:])
                    st2 = stg.tile([128, RT], f16, tag="st2", name="st2")
                    nc.vector.tensor_scalar(st2[:], p2[:], 1.0, None,
                                            Alu.mult)
                    nc.sync.dma_start(out0[128:192, l0:l0 + RT], st2[0:64, :])
                    nc.sync.dma_start(out1[128:192, l0:l0 + RT],
                                      st2[64:128, :])

            # ================= schedule =================
            ident_t = wp.tile([128, 128], f16, tag="ident", name="ident_t")
            nc.sync.dma_start(ident_t[:], ident[:, :])

            with tc.tile_pool(name="psA", bufs=2, space="PSUM") as psA, \
                 tc.tile_pool(name="psB", bufs=1, space="PSUM") as _psB:
                psBd["p"] = _psB
                pool_dve("P0", xa0)
                pool_dve("P2", xb)
                a_plane("P0")
                kgen0()
                build_diag("P0")
                xpo0 = make_xpo("P0")
                a_plane("P2")
                pool_act("P1", xa1)
                a_plane("P1")
                kgen1()
                build_diag("P2")
                build_diag("P1")
                xpo2 = make_xpo("P2")

            with tc.tile_pool(name="psC", bufs=2, space="PSUM") as psC:
                y_store["P2"] = xa0
                phase_c_dve("P2", xpo2, 0, R_DVE["P2"])
                phase_c_pe("P2", R_DVE["P2"], H)
                phase_c_dve("P0", xpo0, 0, R_DVE["P0"])
                phase_c_pe("P0", R_DVE["P0"], H)
                phase_c_fixups("P2", 0, H)
                phase_c_fixups("P0", 0, H)
                y_store["P1"] = xa1
                xpo1 = make_xpo("P1")
                phase_c_dve("P1", xpo1, 0, R_DVE["P1"])
                phase_c_pe("P1", R_DVE["P1"], H)
                phase_c_fixups("P1", 0, H)

            with tc.tile_pool(name="psD", bufs=2, space="PSUM") as psD:
                d_img0()
                d_img1()

    nc.compile()
    return nc


def _get_nc():
    if "nc" not in _BUILT:
        _BUILT["nc"] = build()
    return _BUILT["nc"]


def _host_prep(Wk, bk, Wg, bg, Wx, bx, Wp, bp, dc):
    f32 = lambda a: np.ascontiguousarray(np.asarray(a, dtype=np.float32))
    f16T = lambda a: np.ascontiguousarray(
        np.asarray(a, dtype=np.float32).T.astype(np.float16))
    WxT = f16T(Wx)     # [c, o]
    WpT = f16T(Wp)
    WkT = f16T(Wk)
    z64 = np.zeros((64, 128), np.float16)

    def streams(WT, pre):
        blk = WT[128:192, 128:192]
        bd = np.zeros((128, 128), np.float16)
        bd[0:64, 0:64] = blk
        bd[64:128, 64:128] = blk
        return {
            f"w_{pre}1": np.ascontiguousarray(WT[0:128, 0:128]),
            f"w_{pre}2_0": np.ascontiguousarray(
                np.concatenate([WT[128:192, 0:128], z64], axis=0)),
            f"w_{pre}2_1": np.ascontiguousarray(
                np.concatenate([z64, WT[128:192, 0:128]], axis=0)),
            f"w_{pre}3": np.ascontiguousarray(WT[0:128, 128:192]),
            f"w_{pre}4": bd,
        }

    colv = lambda v, lo, hi: np.ascontiguousarray(
        np.asarray(v, dtype=np.float32)[lo:hi].reshape(-1, 1))
    dup_col = lambda v: np.ascontiguousarray(
        np.concatenate([colv(v, 128, 192), colv(v, 128, 192)], axis=0))
    zpad0 = lambda wT: np.ascontiguousarray(np.concatenate(
        [wT[128:192], np.zeros((64, 192), wT.dtype)], axis=0))
    zpad1 = lambda wT: np.ascontiguousarray(np.concatenate(
        [np.zeros((64, 192), wT.dtype), wT[128:192]], axis=0))

    shared = {}
    shared.update(streams(WxT, "a"))
    shared.update(streams(WpT, "p"))
    shared.update({
        "wkT_a": np.ascontiguousarray(WkT[0:128]),
        "wkT_b0": zpad0(WkT), "wkT_b1": zpad1(WkT),
        "wg2": np.ascontiguousarray(
            (0.5 * np.asarray(Wg, dtype=np.float32)).T.astype(np.float16)),
        "bx_a": colv(bx, 0, 128), "bx_b": dup_col(bx),
        "dc_a": colv(dc, 0, 128), "dc_b": dup_col(dc),
        "bk_bc": np.ascontiguousarray(np.tile(f32(bk).reshape(1, C), (9, 1))),
        "bg_bc": np.ascontiguousarray(np.tile(f32(bg).reshape(1, 9), (128, 1))),
        "ident": np.ascontiguousarray(np.eye(128, dtype=np.float16)),
    })
    return shared


def kernel(x, Wk, bk, Wg, bg, Wx, bx, Wp, bp, dc):
    nc = _get_nc()
    x16 = np.asarray(x, dtype=np.float32).reshape(B, C, L).astype(np.float16)
    shared = _host_prep(Wk, bk, Wg, bg, Wx, bx, Wp, bp, dc)
    in_maps = []
    for core in range(NCORES):
        m = dict(shared)
        m["x0"] = np.ascontiguousarray(x16[2 * core])
        m["x1"] = np.ascontiguousarray(x16[2 * core + 1])
        in_maps.append(m)

    res = bass_utils.run_bass_kernel_spmd(nc, in_maps,
                                          core_ids=list(range(NCORES)))
    bpc = np.asarray(bp, dtype=np.float32).reshape(1, C, 1)
    out = np.empty((B, C, H, W), dtype=np.float32)
    for core in range(NCORES):
        o0 = res.results[core]["out0"].astype(np.float32) + bpc[0]
        o1 = res.results[core]["out1"].astype(np.float32) + bpc[0]
        out[2 * core] = o0.reshape(C, H, W)
        out[2 * core + 1] = o1.reshape(C, H, W)
    return out
